# revision 20
# baseline (speedup 1.0000x reference)
"""GIN message-passing GNN (6 layers) on 8 Trainium2 NeuronCores.

Sharding: edges partitioned by dst into 8 node-shards (6250 dst rows each).
Each core: dma_gather(h[src]) for its edges -> one-hot matmul segment-sum per
128-dst block (+ self term via contiguous-slab PE transpose-accumulate) ->
MLP in feature-major layout -> transposed write of its h shard -> AllGather
replicates h for the next layer.

vs. v1 baseline (7.70ms): layer-0 slots are pre-gathered on the host and
streamed with plain DMA (no gpsimd), per-block exact ktile counts replace the
uniform global max (less padding), the N self-edges are dropped from the
gather (slab matmul instead), and h lives in two chunk-major shared buffers
(local rows [0,4096) and [4096,6250) across cores, both < 32768 rows so int16
gather indices reach them directly) which lets each AllGather chunk fire as
soon as its blocks are written and overlap the layer tail.

Self-contained: hardcodes all shapes; builds/compiles the Bass program on
first call, specialized to the runtime edge structure.
"""
import math
import sys

import numpy as np

sys.path.insert(0, "/opt/trn_rl_repo")
sys.path.insert(0, "/root/problem")

N, E = 50000, 800000
IN, H, OUT = 128, 128, 64
N_MID = 4
NCORES = 8
SHARD = N // NCORES          # 6250
NBLK = math.ceil(SHARD / 128)  # 49 blocks per core (last = 106 rows)
LAST_ROWS = SHARD - 128 * (NBLK - 1)
CHUNK_R = 4096               # local-row chunk boundary: chunk0 = 8*4096 = 32768
CH0 = NCORES * CHUNK_R       # rows in h chunk-0 buffer (int16 idx limit)
CH1 = NCORES * (SHARD - CHUNK_R)  # rows in h chunk-1 buffer (17232)
MAX_G = 8                    # max ktiles (1024 idxs) per dma_gather call (HW limit)


def _wrap_idxs_flat(idx_vals: np.ndarray) -> np.ndarray:
    """[n] -> [128, n//16] int16 (16-partition wrap, replicated 8x)."""
    n = len(idx_vals)
    assert n % 16 == 0
    buf = np.zeros((16, n // 16), dtype=np.int16)
    ar = np.arange(n)
    buf[ar % 16, ar // 16] = idx_vals.astype(np.int16)
    return np.tile(buf, (8, 1))


def _prep_edges(edge_index: np.ndarray, x: np.ndarray):
    """Partition/sort/pad edges (no self edges - handled by slab matmul).

    Returns per-core idx tables, dstrel tables, per-block (K_LO_b, K_HI_b)
    compile-time counts (max over cores), and per-core layer-0 pre-gathered
    slot tables built from x.
    """
    src = edge_index[0].astype(np.int64)
    dst = edge_index[1].astype(np.int64)
    # chunk-major h layout: node n = (k, r) -> chunk0 row k*4096+r (r<4096)
    # or chunk1 row k*2154+(r-4096); both buffers < 32768 rows (int16 idx)
    owner = src // SHARD
    local = src - owner * SHARD
    in_c0 = local < CHUNK_R
    src_c = np.where(in_c0, owner * CHUNK_R + local,
                     owner * (SHARD - CHUNK_R) + (local - CHUNK_R))
    core_of = dst // SHARD
    per_core = []
    klo = np.zeros(NBLK, dtype=np.int64)
    khi = np.zeros(NBLK, dtype=np.int64)
    for k in range(NCORES):
        m = core_of == k
        s, sc, c0, d = src[m], src_c[m], in_c0[m], dst[m] - k * SHARD
        blk = d // 128
        lo_lists, hi_lists = [], []
        for b in range(NBLK):
            mb_ = blk == b
            sb, scb, c0b, db = s[mb_], sc[mb_], c0[mb_], d[mb_] - 128 * b
            lo_lists.append((scb[c0b], db[c0b], sb[c0b]))
            hi_lists.append((scb[~c0b], db[~c0b], sb[~c0b]))
            klo[b] = max(klo[b], math.ceil(max(int(c0b.sum()), 1) / 128))
            khi[b] = max(khi[b], math.ceil(max(int((~c0b).sum()), 1) / 128))
        per_core.append((lo_lists, hi_lists))

    kt = klo + khi               # per-block ktiles
    blk_off = np.concatenate([[0], np.cumsum(kt)])  # ktile offset per block
    total_kt = int(blk_off[-1])

    idx_tables, dstrel_tables, xg_tables = [], [], []
    for k in range(NCORES):
        lo_lists, hi_lists = per_core[k]
        idx_flat = np.zeros(total_kt * 128, dtype=np.int64)
        # padding slots: idx 0 (gathers junk), dstrel -1 (one-hot zero)
        rel_flat = np.full(total_kt * 128, -1.0, dtype=np.float32)
        src_flat = np.zeros(total_kt * 128, dtype=np.int64)  # true node ids
        for b in range(NBLK):
            base = int(blk_off[b]) * 128
            for (scb, db, sgb), off in (
                (lo_lists[b], 0),
                (hi_lists[b], int(klo[b]) * 128),
            ):
                n = len(scb)
                idx_flat[base + off: base + off + n] = scb
                rel_flat[base + off: base + off + n] = db.astype(np.float32)
                src_flat[base + off: base + off + n] = sgb
        idx_tables.append(_wrap_idxs_flat(idx_flat))
        # dstrel layout [128, total_kt]: slot j -> partition j%128, col j//128
        dstrel_tables.append(rel_flat.reshape(total_kt, 128).T.copy())
        # layer-0 pre-gathered slots, slot-major [total_kt*128, IN] f32
        xg_tables.append(np.ascontiguousarray(x[src_flat]))
    return (idx_tables, dstrel_tables, xg_tables,
            tuple(int(v) for v in klo), tuple(int(v) for v in khi))


_CACHE = {}


def _build(klo, khi):
    from concourse import bacc, mybir, library_config
    from concourse.tile import TileContext

    kt = [a + b for a, b in zip(klo, khi)]
    blk_off = [0]
    for v in kt:
        blk_off.append(blk_off[-1] + v)
    total_kt = blk_off[-1]
    max_kt = max(kt)

    nc = bacc.Bacc("TRN2", target_bir_lowering=False, debug=False,
                   num_devices=NCORES)
    f32 = mybir.dt.float32

    xs_in = nc.declare_dram_parameter("xs", [SHARD, IN], f32, isOutput=False)
    xg_in = nc.declare_dram_parameter("xg", [total_kt * 128, IN], f32, isOutput=False)
    idxs_in = nc.declare_dram_parameter("idxs", [128, total_kt * 8], mybir.dt.int16, isOutput=False)
    dstrel_in = nc.declare_dram_parameter("dstrel", [128, total_kt], f32, isOutput=False)
    iota_in = nc.declare_dram_parameter("iota", [128, 128], f32, isOutput=False)
    ident_in = nc.declare_dram_parameter("ident", [128, 128], f32, isOutput=False)
    wa_in = nc.declare_dram_parameter("wa", [5, 128, 128], f32, isOutput=False)
    wb_in = nc.declare_dram_parameter("wb", [5, 128, 128], f32, isOutput=False)
    ba_in = nc.declare_dram_parameter("ba", [5, 128], f32, isOutput=False)
    bb_in = nc.declare_dram_parameter("bb", [5, 128], f32, isOutput=False)
    wl_in = nc.declare_dram_parameter("wl", [128, OUT], f32, isOutput=False)
    bl_in = nc.declare_dram_parameter("bl", [OUT], f32, isOutput=False)
    out_ext = nc.declare_dram_parameter("out", [SHARD, OUT], f32, isOutput=True)

    ag_in = nc.dram_tensor("ag_in", [SHARD, H], f32)
    hbufs = [(nc.dram_tensor(f"h{i}c0", [CH0, H], f32, addr_space="Shared"),
              nc.dram_tensor(f"h{i}c1", [CH1, H], f32, addr_space="Shared"))
             for i in range(2)]

    with TileContext(nc) as tc:
        with tc.tile_pool(name="cst", bufs=1) as cst, \
             tc.tile_pool(name="gat", bufs=6) as gat, \
             tc.tile_pool(name="slab", bufs=4) as slb, \
             tc.tile_pool(name="ahot", bufs=4) as ahot, \
             tc.tile_pool(name="work", bufs=4) as work, \
             tc.tile_pool(name="psum", bufs=3, space="PSUM") as ps, \
             tc.tile_pool(name="psmlp", bufs=1, space="PSUM") as psm:
            nc.gpsimd.load_library(library_config.mlp)
            idx_t = cst.tile([128, total_kt * 8], mybir.dt.int16)
            nc.sync.dma_start(out=idx_t[:], in_=idxs_in[:, :])
            dstrel_t = cst.tile([128, total_kt], f32)
            nc.sync.dma_start(out=dstrel_t[:], in_=dstrel_in[:, :])
            iota_t = cst.tile([128, 128], f32)
            nc.sync.dma_start(out=iota_t[:], in_=iota_in[:, :])
            ident_t = cst.tile([128, 128], f32)
            nc.sync.dma_start(out=ident_t[:], in_=ident_in[:, :])
            wa_t = cst.tile([128, 5, 128], f32)
            nc.sync.dma_start(out=wa_t[:], in_=wa_in[:, :, :].rearrange("l p d -> p l d"))
            wb_t = cst.tile([128, 5, 128], f32)
            nc.sync.dma_start(out=wb_t[:], in_=wb_in[:, :, :].rearrange("l p d -> p l d"))
            ba_t = cst.tile([128, 5], f32)
            nc.sync.dma_start(out=ba_t[:], in_=ba_in[:, :].rearrange("l p -> p l"))
            bb_t = cst.tile([128, 5], f32)
            nc.sync.dma_start(out=bb_t[:], in_=bb_in[:, :].rearrange("l p -> p l"))
            wl_t = cst.tile([128, OUT], f32)
            nc.sync.dma_start(out=wl_t[:], in_=wl_in[:, :])
            bl_t = cst.tile([OUT, 1], f32)
            nc.sync.dma_start(out=bl_t[:], in_=bl_in[:, None])

            for layer in range(6):
                h_src = (None, None) if layer == 0 else hbufs[(layer - 1) % 2]
                h_dst = hbufs[layer % 2]
                final = layer == 5
                wcols = OUT if final else H

                # chunked AllGather: chunk0 (local rows < 4096) fires after
                # block 31 and overlaps the layer tail; chunk1 at the end
                ag_after = {CHUNK_R // 128 - 1: (0, CHUNK_R, 0),
                            NBLK - 1: (CHUNK_R, SHARD, 1)}

                sc = nc.named_scope(f"L{layer}")
                sc.__enter__()
                for b in range(NBLK):
                    rows = LAST_ROWS if b == NBLK - 1 else 128
                    ktb = kt[b]
                    kt_base = blk_off[b]
                    g_t = gat.tile([128, max_kt, H], f32, tag="gt")
                    if layer == 0:
                        # stream pre-gathered x slots: plain DMA, no gpsimd
                        nc.sync.dma_start(
                            out=g_t[:, :ktb, :],
                            in_=xg_in[kt_base * 128:(kt_base + ktb) * 128, :]
                                .rearrange("(k p) f -> p k f", p=128))
                    else:
                        for part_off, part_kt, h_ch in (
                                (0, klo[b], h_src[0]), (klo[b], khi[b], h_src[1])):
                            done = 0
                            while done < part_kt:
                                cnt = min(MAX_G, part_kt - done)
                                co = kt_base + part_off + done
                                nc.gpsimd.dma_gather(
                                    g_t[:, part_off + done: part_off + done + cnt, :],
                                    h_ch[:, :],
                                    idx_t[:, co * 8:(co + cnt) * 8],
                                    cnt * 128, cnt * 128, H)
                                done += cnt
                    # self-term slab: this core's own shard rows. L0: host-
                    # sliced xs input; L1-5: ag_in (the pre-AllGather output
                    # this core wrote last layer == its shard of h).
                    slab_t = slb.tile([128, H], f32, tag="slab")
                    row0 = b * 128
                    slab_src = xs_in if layer == 0 else ag_in
                    nc.sync.dma_start(out=slab_t[:rows, :],
                                      in_=slab_src[row0:row0 + rows, :])
                    # --- one-hot build (one DVE op) ---
                    a_t = ahot.tile([128, max_kt, 128], f32, tag="at")
                    nc.vector.tensor_tensor(
                        out=a_t[:, :ktb, :],
                        in0=iota_t[:, None, :].to_broadcast([128, ktb, 128]),
                        in1=dstrel_t[:, kt_base:kt_base + ktb, None].to_broadcast([128, ktb, 128]),
                        op=mybir.AluOpType.is_equal)
                    # --- aggregation psum: agg[feat, dst] ---
                    agg_p = ps.tile([128, 128], f32, tag="agg")
                    # self term: agg[feat, node] += slab[node, feat]; full-
                    # width rhs zeroes cols >= rows so start=True covers all
                    nc.tensor.matmul(out=agg_p[:], lhsT=slab_t[:rows, :],
                                     rhs=ident_t[:rows, :],
                                     start=True, stop=False)
                    for kk in range(ktb):
                        nc.tensor.matmul(out=agg_p[:], lhsT=g_t[:, kk, :], rhs=a_t[:, kk, :],
                                         start=False, stop=(kk == ktb - 1))
                    aggT = work.tile([128, 128], f32, tag="aggT")
                    nc.vector.tensor_copy(out=aggT[:], in_=agg_p[:])
                    # --- MLP ---
                    if final:
                        z_p = psm.tile([128, 128], f32, tag="z1")
                        nc.tensor.matmul(out=z_p[:OUT, :], lhsT=wl_t[:], rhs=aggT[:],
                                         start=True, stop=True)
                        z_t = work.tile([128, 128], f32, tag="zt")
                        nc.scalar.activation(out=z_t[:OUT, :], in_=z_p[:OUT, :],
                                             func=mybir.ActivationFunctionType.Sigmoid,
                                             bias=bl_t[:], scale=1.0)
                    else:
                        t1_p = psm.tile([128, 128], f32, tag="z1")
                        nc.tensor.matmul(out=t1_p[:], lhsT=wa_t[:, layer, :], rhs=aggT[:],
                                         start=True, stop=True)
                        t1 = work.tile([128, 128], f32, tag="t1")
                        nc.scalar.activation(out=t1[:], in_=t1_p[:],
                                             func=mybir.ActivationFunctionType.Relu,
                                             bias=ba_t[:, layer, None], scale=1.0)
                        z2_p = psm.tile([128, 128], f32, tag="z2")
                        nc.tensor.matmul(out=z2_p[:], lhsT=wb_t[:, layer, :], rhs=t1[:],
                                         start=True, stop=True)
                        z_t = work.tile([128, 128], f32, tag="zt")
                        nc.scalar.activation(out=z_t[:], in_=z2_p[:],
                                             func=mybir.ActivationFunctionType.Relu,
                                             bias=bb_t[:, layer, None], scale=1.0)
                    # --- transpose z -> node-major, write out ---
                    zT_p = psm.tile([128, 128], f32, tag="zT")
                    nc.tensor.matmul(out=zT_p[:, :wcols], lhsT=z_t[:wcols, :], rhs=ident_t[:wcols, :wcols],
                                     start=True, stop=True)
                    zz = work.tile([128, 128], f32, tag="zz")
                    nc.vector.tensor_copy(out=zz[:, :wcols], in_=zT_p[:, :wcols])
                    if final:
                        nc.sync.dma_start(out=out_ext[row0:row0 + rows, :], in_=zz[:rows, :OUT])
                    else:
                        nc.sync.dma_start(out=ag_in[row0:row0 + rows, :], in_=zz[:rows, :H])
                        if b in ag_after:
                            r0, r1, ci = ag_after[b]
                            with nc.named_scope(f"AG{layer}c{ci}"):
                                nc.gpsimd.collective_compute(
                                    "AllGather", mybir.AluOpType.bypass,
                                    replica_groups=[list(range(NCORES))],
                                    ins=[ag_in[r0:r1, :]],
                                    outs=[h_dst[ci][:, :]])
                sc.__exit__(None, None, None)
    nc.compile()
    return nc


def kernel(**inputs):
    from concourse.bass_utils import run_bass_kernel_spmd

    x = np.asarray(inputs["x"], np.float32)
    edge_index = np.asarray(inputs["edge_index"])
    idx_tables, dstrel_tables, xg_tables, klo, khi = _prep_edges(edge_index, x)

    key = (klo, khi)
    if key not in _CACHE:
        _CACHE[key] = _build(klo, khi)
    nc = _CACHE[key]

    wa = np.stack([inputs["w0a"]] + [inputs["wma"][i] for i in range(N_MID)]).astype(np.float32)
    wb = np.stack([inputs["w0b"]] + [inputs["wmb"][i] for i in range(N_MID)]).astype(np.float32)
    ba = np.stack([inputs["b0a"]] + [inputs["bma"][i] for i in range(N_MID)]).astype(np.float32)
    bb = np.stack([inputs["b0b"]] + [inputs["bmb"][i] for i in range(N_MID)]).astype(np.float32)

    iota = np.tile(np.arange(128, dtype=np.float32), (128, 1))
    ident = np.eye(128, dtype=np.float32)
    in_maps = []
    for k in range(NCORES):
        in_maps.append({
            "xs": np.ascontiguousarray(x[k * SHARD:(k + 1) * SHARD]),
            "xg": xg_tables[k], "idxs": idx_tables[k],
            "dstrel": dstrel_tables[k],
            "iota": iota, "ident": ident,
            "wa": wa, "wb": wb, "ba": ba, "bb": bb,
            "wl": inputs["wl"].astype(np.float32), "bl": inputs["bl"].astype(np.float32),
        })
    kernel._last = (nc, in_maps)  # test.py hook for traced re-runs
    res = run_bass_kernel_spmd(nc, in_maps, core_ids=list(range(NCORES)))
    out = np.concatenate([res.results[k]["out"] for k in range(NCORES)], axis=0)
    return out.astype(np.float32)
